# revision 25
# baseline (speedup 1.0000x reference)
"""Trainium2 Bass kernel for a 2-layer ReLU RNN (batch_first) + linear head.

Problem shapes: B=256, T=512, I=512, H=1024, O=256 (fp32).
Sharding: data-parallel over batch across 8 NeuronCores (32 rows each);
weights replicated. No collectives.

Per-core design (all matmul operands bf16, fp32 PSUM accumulate):

  Both layers' input projections are BATCHED full-array GEMMs; only the
  recurrences run in-step:
    L0 step t: s0 = pre0_t + h0 @ W_hh0.T   (8 k-tiles in-step)
    L1 step t: s1 = pre1_t + h1 @ W_hh1.T   (8 k-tiles in-step)
    pre0 = x @ W_ih0.T + b0, pre1 = h0 @ W_ih1.T + b1 computed as
    "parts": full-array (stationary = 128x128 weight block) matmuls over
    half-chunk moving windows (N=256 = 8 steps x 32 batch), bias folded
    in the PSUM->SBUF copy, writing pre{0,1}T directly in hT layout.

  In-step s-matmuls run 4x column-tiled (tile_size 128x32): col-tile v
  computes the interleaved output column set {j : (j//32)%4 == v}. This
  makes the PSUM layout ps[32v+b, 32w+j'] = s[b, 128w+32v+j'], which is
  32x32-block-transpose compatible: DVE StreamTranspose halves of
  [128, 256] yield hT[jj, 32kt+b], so the PE does no transposes.
  Both chains: DVE transpose (psum) -> DVE add(+preT) -> ACT relu,
  executing under the other sections' matmul windows.

  Schedule: every step carries [L0 8 rounds][L1 8 rounds][one L1
  quarter-part (8 MMs N=256) + one x quarter-part (4 MMs N=256)].
  Uniform per-step phase spreading covers the state-chain latency on
  every step (8-of-16 spreading measured ~540 ns/step of chain stalls);
  merging both projections into ONE full-array section per step keeps
  the col-tiled<->full-array transitions at 2/step, and batching the
  x-proj (vs in-step x rounds) cuts LDWEIGHTS traffic from 3584 to
  3328 cols/step, giving the saturated 1.2 GHz weight-load port slack.
  x parts for chunk c run during chunk c-1 (xt ring prefetch 24 steps);
  L1 parts for chunk c run at steps c*16+8..+23, so L1 lags L0 by 24.

  Measured floor context: in-step rounds and full-array N=256 MMs both
  issue at ~109 ns warm; 4-way col tiling is the LDW/stream balance
  point; one LDW cannot feed multiple col groups (verified); DoubleRow
  fp8 is col-tiling-incompatible and accuracy-breaking.

  Fill/drain: weight/const DMAs ride the gpsimd SWDGE queue so the sync
  queue carries only the (contiguous 32 KB) xt stream; chunk-0 x parts
  run pre-loop under the weight DMA window.  Drain L1 steps use
  quartered (64-col) chains and are interleaved with the last chunk's
  parts then discarded filler matmuls -- without filler the PE idles
  >3.4 us/step and HAM re-throttles it to 1.2 GHz (measured cold-clock
  MM durations across the whole drain).

kernel(**inputs) takes the FULL unsharded inputs (keys as in the
reference setup_inputs) and returns the FULL [256, 256] output.
"""

import ml_dtypes
import numpy as np

import concourse.bass as bass
import concourse.tile as tile
import concourse.mybir as mybir
from concourse import bacc
from concourse.bass_utils import run_bass_kernel_spmd

F32 = mybir.dt.float32
BF16 = mybir.dt.bfloat16

B_FULL, T_FULL, I_DIM, H, O = 256, 512, 512, 1024, 256
N_CORES = 8
BL = B_FULL // N_CORES  # 32 batch rows per core
KX = I_DIM // 128       # 4 k-tiles of the input dim
KH = H // 128           # 8 k-tiles of the hidden dim
CH = 16                 # chunk length (steps)
LAG = CH + 8            # L1 lags L0 by 24 steps
PREF = 24               # xt DMA prefetch depth (steps ahead)
XS = 2 * CH             # xt ring slots


def _emit_step_mms(nc, ps, stat_tiles, w_sb, kb0, n_k):
    """One recurrence step's s-matmuls, 4x column-tiled.

    ps [128,256] f32 psum; stat_tiles: n_k stationary APs [128,32] bf16;
    w_sb k-block (kb0+i) columns [(kb0+i)*1024 + j] hold W[j, 128*i + r].
    Col-tile v streams columns {j : (j//32)%4 == v} via a strided AP.
    """
    for i in range(n_k):
        blk = w_sb[:, (kb0 + i) * 1024 : (kb0 + i + 1) * 1024].rearrange(
            "p (w f j) -> p w f j", f=4, j=32
        )
        for v in range(4):
            nc.tensor.matmul(
                ps[32 * v : 32 * v + 32, :],
                stat_tiles[i],
                blk[:, :, v : v + 1, :],
                start=(i == 0),
                stop=(i == n_k - 1),
                tile_position=(0, 32 * v),
                skip_group_check=True,
            )


def build_rnn(T):
    assert T % CH == 0
    nc = bacc.Bacc("TRN2", target_bir_lowering=False, debug=False)

    # x is pre-laid-out per step: row block t*128..t*128+128 is step t's
    # tile [128, KX*BL] = x^T in (ki, b) column order -- one contiguous
    # 32 KB block per step -> a single dense DMA burst (the old [I, T*BL]
    # gather produced 64 B scattered elements that crawled when
    # contending with the weight stream).
    xTb_d = nc.dram_tensor("xTb", [T * 128, KX * BL], BF16, kind="ExternalInput").ap()
    w0_d = nc.dram_tensor("w0cat", [128, (KX + KH) * H], BF16, kind="ExternalInput").ap()
    w1_d = nc.dram_tensor("w1cat", [128, (KH + KH) * H], BF16, kind="ExternalInput").ap()
    fcw_d = nc.dram_tensor("fcwT", [128, KH * O], BF16, kind="ExternalInput").ap()
    b0_d = nc.dram_tensor("bias0pp", [128, KH], F32, kind="ExternalInput").ap()
    b1_d = nc.dram_tensor("bias1pp", [128, KH], F32, kind="ExternalInput").ap()
    fcb_d = nc.dram_tensor("fcb", [BL, O], F32, kind="ExternalInput").ap()
    out_d = nc.dram_tensor("out", [BL, O], F32, kind="ExternalOutput").ap()

    with tile.TileContext(nc) as tc:
        with (
            tc.tile_pool(name="wpool", bufs=1) as wpool,
            tc.tile_pool(name="cpool", bufs=1) as cpool,
            tc.tile_pool(name="hT1", bufs=3) as hT1_pool,
            tc.tile_pool(name="tr0", bufs=2) as tr0_pool,
            tc.tile_pool(name="ad0", bufs=2) as ad0_pool,
            tc.tile_pool(name="tr1", bufs=2) as tr1_pool,
            tc.tile_pool(name="ad1", bufs=2) as ad1_pool,
            tc.tile_pool(name="ps0", bufs=2, space="PSUM") as ps0_pool,
            tc.tile_pool(name="ps1", bufs=2, space="PSUM") as ps1_pool,
            tc.tile_pool(name="psc", bufs=3, space="PSUM") as psc_pool,
            tc.tile_pool(name="psh", bufs=1, space="PSUM") as psh_pool,
            tc.tile_pool(name="eout", bufs=1) as eo_pool,
        ):
            w0_sb = wpool.tile([128, (KX + KH) * H], BF16)
            w1_sb = wpool.tile([128, (KH + KH) * H], BF16)
            fcw_sb = wpool.tile([128, KH * O], BF16)
            b0_sb = cpool.tile([128, KH], F32)
            b1_sb = cpool.tile([128, KH], F32)
            fcb_sb = cpool.tile([BL, O], F32)
            # h0T ring: 2*CH slots of [128, 256]; slot u%(2CH) = step u's h0T
            ring = cpool.tile([128, 2 * CH * 256], BF16)
            # pre0T / pre1T double rings: 2 chunks x 16 slots of [128, 256]
            ring0 = cpool.tile([128, 2 * CH * 256], BF16)
            ring2 = cpool.tile([128, 2 * CH * 256], BF16)
            # xt ring: XS slots of [128, 128] (step t -> slot t%XS)
            ringx = cpool.tile([128, XS * 128], BF16)

            # Weight/const DMAs on the gpsimd SWDGE queue; sync queue
            # carries only the xt stream.  Issue order = need order.
            nc.gpsimd.dma_start(b0_sb[:], b0_d)
            # w0's x-blocks split per k-tile: the prologue x-quarters gate
            # on their own kb block, not the full 1 MB.
            for kb in range(KX):
                nc.gpsimd.dma_start(w0_sb[:, kb * H : (kb + 1) * H],
                                    w0_d[:, kb * H : (kb + 1) * H])
            nc.gpsimd.dma_start(w0_sb[:, KX * H : (KX + 4) * H],
                                w0_d[:, KX * H : (KX + 4) * H])
            nc.gpsimd.dma_start(w0_sb[:, (KX + 4) * H :], w0_d[:, (KX + 4) * H :])
            nc.gpsimd.dma_start(b1_sb[:], b1_d)
            nc.gpsimd.dma_start(w1_sb[:, : KH * H], w1_d[:, : KH * H])
            nc.gpsimd.dma_start(w1_sb[:, KH * H :], w1_d[:, KH * H :])
            nc.gpsimd.dma_start(fcw_sb[:], fcw_d)
            nc.gpsimd.dma_start(fcb_sb[:], fcb_d)

            def emit_xt_dma(t, n=1):
                # [t, t+n) must not wrap the ring (n=1 always safe).
                s = t % XS
                nc.sync.dma_start(
                    ringx[:, s * 128 : (s + n) * 128],
                    xTb_d[t * 128 : (t + n) * 128, :].rearrange(
                        "(t p) c -> p t c", p=128
                    ),
                )

            rx_view = ringx.rearrange("p (t k b) -> p t k b", t=XS, b=32)
            r0_view = ring0.rearrange("p (h t k b) -> p h t k b", h=2, t=CH, b=32)
            r2_view = ring2.rearrange("p (h t k b) -> p h t k b", h=2, t=CH, b=32)
            rh_view = ring.rearrange("p (t k b) -> p t k b", t=2 * CH, b=32)

            def x_quarter(c, jb, hc, dve_copy=False):
                """pre0T jb-block for the hc-th half (8 steps) of chunk c:
                x^T chunk from the xt ring streamed against stationary
                W_ih0 jb-blocks; bias0 folded in the copy (ACT in steady
                state; the prologue alternates ACT/DVE so the 16
                back-to-back quarters don't serialize on one engine)."""
                t0 = (c % 2) * CH + 8 * hc
                pc = psc_pool.tile([128, 256], F32, tag="psc")
                for kb in range(KX):
                    nc.tensor.matmul(
                        pc[:, :],
                        w0_sb[:, kb * 1024 + 128 * jb : kb * 1024 + 128 * jb + 128],
                        rx_view[:, t0 : t0 + 8, kb : kb + 1, :],
                        start=(kb == 0),
                        stop=(kb == KX - 1),
                        tile_position=(0, 0),
                        skip_group_check=True,
                    )
                out_ap = r0_view[:, c % 2 : c % 2 + 1, 8 * hc : 8 * hc + 8,
                                 jb : jb + 1, :]
                in_ap = pc[:, :].rearrange("p (t b) -> p t b", b=32)
                if dve_copy:
                    nc.vector.tensor_scalar_add(
                        out_ap, in_ap, b0_sb[:, jb : jb + 1]
                    )
                else:
                    nc.scalar.activation(
                        out_ap, in_ap,
                        mybir.ActivationFunctionType.Identity,
                        bias=b0_sb[:, jb : jb + 1],
                    )

            def phase_c_part(c, jb, hc):
                """pre1T jb-block for the hc-th half (8 steps) of chunk c:
                batched GEMM from the h0T ring (half (c%2)); bias1 folded
                in the PSUM->SBUF copy on ACT."""
                t0 = 8 * hc
                half_base = (c % 2) * CH
                pc = psc_pool.tile([128, 256], F32, tag="psc")
                for kb in range(KH):
                    nc.tensor.matmul(
                        pc[:, :],
                        w1_sb[:, kb * 1024 + 128 * jb : kb * 1024 + 128 * jb + 128],
                        rh_view[:, half_base + t0 : half_base + t0 + 8,
                                kb : kb + 1, :],
                        start=(kb == 0),
                        stop=(kb == KH - 1),
                        tile_position=(0, 0),
                        skip_group_check=True,
                    )
                out_ap = r2_view[:, c % 2 : c % 2 + 1, t0 : t0 + 8, jb : jb + 1, :]
                in_ap = pc[:, :].rearrange("p (t b) -> p t b", b=32)
                nc.scalar.activation(
                    out_ap, in_ap,
                    mybir.ActivationFunctionType.Identity,
                    bias=b1_sb[:, jb : jb + 1],
                )

            # L1-part schedule: part (c, jb, hc) needs h0T of steps
            # c*CH + 8*hc..+7; it runs at step u = c*CH + 8 + 8*hc + jb.
            # All 16 parts of chunk c are done by step c*CH + 23, in time
            # for the first chunk-c L1 step at u = c*CH + LAG.
            def part_for_step(u):
                g = u - 8
                c, r = g // CH, g % CH
                jb, hc = (r, 0) if r < 8 else (r - 8, 1)
                return c, jb, hc

            def l0_step(u, fine=False):
                if u == 0:
                    # h0_0 = relu(pre0_0): no matmul.
                    nc.scalar.activation(
                        ring[:, 0:256],
                        ring0[:, 0:256],
                        mybir.ActivationFunctionType.Relu,
                    )
                    return
                s_prev = 256 * ((u - 1) % (2 * CH))
                prev = ring[:, s_prev : s_prev + 256]
                stats = [prev[:, 32 * k : 32 * k + 32] for k in range(KH)]
                ps0 = ps0_pool.tile([128, 256], F32, tag="ps0")
                _emit_step_mms(nc, ps0, stats, w0_sb, KX, KH)
                # chain: transpose (psum f32 -> sbuf f32) -> +pre0T -> relu
                off0 = ((u // CH) % 2) * (CH * 256) + 256 * (u % CH)
                pre0 = ring0[:, off0 : off0 + 256]
                tr = ring[:, 256 * (u % (2 * CH)) : 256 * (u % (2 * CH)) + 256]
                trt = tr0_pool.tile([128, 256], F32, tag="tr0")
                ad = ad0_pool.tile([128, 256], BF16, tag="ad0")
                W = 64 if fine else 128
                for h in range(256 // W):
                    cs = slice(W * h, W * h + W)
                    nc.vector.transpose(trt[:, cs], ps0[:, cs])
                    nc.vector.tensor_add(ad[:, cs], trt[:, cs], pre0[:, cs])
                    nc.scalar.activation(
                        tr[:, cs], ad[:, cs],
                        mybir.ActivationFunctionType.Relu,
                    )

            hT1 = None

            def l1_step(u, fine=False):
                nonlocal hT1
                off = ((u // CH) % 2) * (CH * 256) + 256 * (u % CH)
                pre1 = ring2[:, off : off + 256]
                if u == 0:
                    hT1_new = hT1_pool.tile([128, 256], BF16, tag="hT1")
                    nc.scalar.activation(
                        hT1_new[:, :], pre1, mybir.ActivationFunctionType.Relu
                    )
                    hT1 = hT1_new
                    return
                stats = [hT1[:, 32 * k : 32 * k + 32] for k in range(KH)]
                ps1 = ps1_pool.tile([128, 256], F32, tag="ps1")
                _emit_step_mms(nc, ps1, stats, w1_sb, KH, KH)
                # chain: transpose -> +pre1T -> relu; fine=True quarters
                # the slices so the serial drain steps expose ~1/4 of the
                # chain latency.
                tr = tr1_pool.tile([128, 256], F32, tag="tr1")
                ad = ad1_pool.tile([128, 256], BF16, tag="ad1")
                hT1_new = hT1_pool.tile([128, 256], BF16, tag="hT1")
                W = 64 if fine else 128
                for h in range(256 // W):
                    cs = slice(W * h, W * h + W)
                    nc.vector.transpose(tr[:, cs], ps1[:, cs])
                    nc.vector.tensor_add(ad[:, cs], tr[:, cs], pre1[:, cs])
                    nc.scalar.activation(
                        hT1_new[:, cs], ad[:, cs],
                        mybir.ActivationFunctionType.Relu,
                    )
                hT1 = hT1_new

            def warm_filler():
                """8 discarded full-array matmuls (~880 ns): keeps HAM from
                re-throttling the PE to 1.2 GHz across the serial drain
                steps and covers the hT1 chain latency like a part would."""
                pw = psc_pool.tile([128, 256], F32, tag="psc")
                for kb in range(KH):
                    nc.tensor.matmul(
                        pw[:, :],
                        w1_sb[:, kb * 1024 : kb * 1024 + 128],
                        w1_sb[:, :256],
                        start=True,
                        stop=True,
                        tile_position=(0, 0),
                        skip_group_check=True,
                    )

            # prologue: prefetch the xt ring (two bulk DMAs -- 24 singles
            # would serialize ~15 us of issue time on the sync queue),
            # then compute chunk 0's pre0 under the weight-DMA window
            # (also warms the PE).
            if T >= PREF:
                emit_xt_dma(0, CH)
                emit_xt_dma(CH, PREF - CH)
            else:
                emit_xt_dma(0, T)
            for q in range(16):
                x_quarter(0, q % 8, q // 8, dve_copy=(q % 2 == 1))

            for u in range(T):
                if u + PREF < T:
                    emit_xt_dma(u + PREF)
                l0_step(u, fine=(u < LAG + 2))
                if u >= LAG:
                    l1_step(u - LAG)
                if u >= 8:
                    phase_c_part(*part_for_step(u))
                if u // CH + 1 < T // CH:
                    r = u % CH
                    x_quarter(u // CH + 1, r % 8, r // 8)
            # drain: the last chunk's hc=1 L1-parts interleave with the
            # first 8 tail L1 steps; afterwards discarded filler matmuls
            # keep the PE warm and cover the quartered hT1 chain latency.
            for i, u in enumerate(range(T - LAG, T)):
                l1_step(u, fine=True)
                if i < 8:
                    phase_c_part(T // CH - 1, i, 1)
                elif i < LAG - 1:
                    warm_filler()

            # ---- head: out = h1_last @ fc_w.T + fc_b ----
            hps = psh_pool.tile([BL, O], F32)
            for kb in range(KH):
                nc.tensor.matmul(
                    hps[:, :],
                    hT1[:, 32 * kb : 32 * kb + 32],
                    fcw_sb[:, kb * O : (kb + 1) * O],
                    start=(kb == 0),
                    stop=(kb == KH - 1),
                    tile_position=(0, 0),
                    skip_group_check=True,
                )
            eo = eo_pool.tile([BL, O], F32)
            nc.vector.tensor_add(eo[:, :], hps[:, :], fcb_sb[:, :])
            nc.sync.dma_start(out_d, eo[:, :])

    nc.compile()
    return nc


def _stackT(W, n_k):
    """[128, n_k*cols] bf16: [r, kb*cols + j] = W[j, 128*kb + r]."""
    cols = W.shape[0]
    WT = np.ascontiguousarray(np.asarray(W, np.float32).T)  # [in, out]
    out = np.empty((128, n_k * cols), np.float32)
    for k in range(n_k):
        out[:, k * cols : (k + 1) * cols] = WT[128 * k : 128 * (k + 1), :]
    return out.astype(ml_dtypes.bfloat16)


def _prep_core_inputs(inputs, T):
    f32 = np.float32
    w0cat = np.concatenate(
        [_stackT(np.asarray(inputs["W_ih0"], f32), KX),
         _stackT(np.asarray(inputs["W_hh0"], f32), KH)], axis=1)
    w1cat = np.concatenate(
        [_stackT(np.asarray(inputs["W_ih1"], f32), KH),
         _stackT(np.asarray(inputs["W_hh1"], f32), KH)], axis=1)
    b0 = (np.asarray(inputs["b_ih0"], f32) + np.asarray(inputs["b_hh0"], f32))
    b1 = (np.asarray(inputs["b_ih1"], f32) + np.asarray(inputs["b_hh1"], f32))
    shared = {
        "w0cat": np.ascontiguousarray(w0cat),
        "w1cat": np.ascontiguousarray(w1cat),
        "fcwT": _stackT(np.asarray(inputs["fc_w"], f32), KH),
        "bias0pp": np.ascontiguousarray(b0.reshape(KH, 128).T),
        "bias1pp": np.ascontiguousarray(b1.reshape(KH, 128).T),
        "fcb": np.tile(np.asarray(inputs["fc_b"], f32)[None, :], (BL, 1)),
    }
    x = np.asarray(inputs["input_data"], f32)  # [B, T, I]
    in_maps = []
    for c in range(N_CORES):
        xs = x[c * BL : (c + 1) * BL, :T, :]  # [BL, T, I]
        # [t, p, ki, b]: xT[t*128+p, ki*32+b] = x[b, t, 128*ki+p] -- each
        # step's tile is one contiguous 32 KB block.
        y = xs.transpose(1, 2, 0).reshape(T, KX, 128, BL).transpose(0, 2, 1, 3)
        xT = np.ascontiguousarray(y).reshape(T * 128, KX * BL).astype(
            ml_dtypes.bfloat16)
        in_maps.append(dict(shared, xTb=xT))
    return in_maps


def run(inputs, trace=False, trace_kwargs=None, T=None):
    if T is None:
        T = np.asarray(inputs["input_data"]).shape[1]
    nc = build_rnn(T)
    in_maps = _prep_core_inputs(inputs, T)
    res = run_bass_kernel_spmd(
        nc, in_maps, list(range(N_CORES)), trace=trace, **(trace_kwargs or {})
    )
    out = np.concatenate([res.results[c]["out"] for c in range(N_CORES)], axis=0)
    return out, res


def kernel(**inputs):
    return run(inputs)[0]


# revision 28
# speedup vs baseline: 1.0031x; 1.0031x over previous
"""Trainium2 Bass kernel for a 2-layer ReLU RNN (batch_first) + linear head.

Problem shapes: B=256, T=512, I=512, H=1024, O=256 (fp32).
Sharding: data-parallel over batch across 8 NeuronCores (32 rows each);
weights replicated. No collectives.

Per-core design (all matmul operands bf16, fp32 PSUM accumulate):

  Both layers' input projections are BATCHED full-array GEMMs; only the
  recurrences run in-step:
    L0 step t: s0 = pre0_t + h0 @ W_hh0.T   (8 k-tiles in-step)
    L1 step t: s1 = pre1_t + h1 @ W_hh1.T   (8 k-tiles in-step)
    pre0 = x @ W_ih0.T + b0, pre1 = h0 @ W_ih1.T + b1 computed as
    "parts": full-array (stationary = 128x128 weight block) matmuls over
    half-chunk moving windows (N=256 = 8 steps x 32 batch), bias folded
    in the PSUM->SBUF copy, writing pre{0,1}T directly in hT layout.

  In-step s-matmuls run 4x column-tiled (tile_size 128x32): col-tile v
  computes the interleaved output column set {j : (j//32)%4 == v}. This
  makes the PSUM layout ps[32v+b, 32w+j'] = s[b, 128w+32v+j'], which is
  32x32-block-transpose compatible: DVE StreamTranspose halves of
  [128, 256] yield hT[jj, 32kt+b], so the PE does no transposes.
  Both chains: DVE transpose (psum) -> DVE add(+preT) -> ACT relu,
  executing under the other sections' matmul windows.

  Schedule: every step carries [L0 8 rounds][L1 8 rounds][one L1
  quarter-part (8 MMs N=256) + one x quarter-part (4 MMs N=256)].
  Uniform per-step phase spreading covers the state-chain latency on
  every step (8-of-16 spreading measured ~540 ns/step of chain stalls);
  merging both projections into ONE full-array section per step keeps
  the col-tiled<->full-array transitions at 2/step, and batching the
  x-proj (vs in-step x rounds) cuts LDWEIGHTS traffic from 3584 to
  3328 cols/step, giving the saturated 1.2 GHz weight-load port slack.
  x parts for chunk c run during chunk c-1 (xt ring prefetch 24 steps);
  L1 parts for chunk c run at steps c*16+8..+23, so L1 lags L0 by 24.

  Measured floor context: in-step rounds and full-array N=256 MMs both
  issue at ~109 ns warm; 4-way col tiling is the LDW/stream balance
  point; one LDW cannot feed multiple col groups (verified); DoubleRow
  fp8 is col-tiling-incompatible and accuracy-breaking.  Steady-state
  step measures ~3.28 us = 3.06 us of matmul sections + 2x ~95 ns
  col-tiled<->full-array mode-switch bubbles (structural: the part's
  128-col LDW cannot pre-load across the boundary; any cycle of the two
  modes pays 2 switches).  N=512 parts on alternating steps measured
  WORSE (2.07 ms): the uncovered steps' chains stall the PE even with
  64-col quartered chains (DVE queuing).  Total 1.78 ms HW exec
  (baseline 1.87-2.23 ms), rel err 4.7e-3.

  Fill/drain: weight/const DMAs ride the gpsimd SWDGE queue so the sync
  queue carries only the (contiguous 32 KB) xt stream; chunk-0 x parts
  run pre-loop under the weight DMA window.  Drain L1 steps use
  quartered (64-col) chains and are interleaved with the last chunk's
  parts then discarded filler matmuls -- without filler the PE idles
  >3.4 us/step and HAM re-throttles it to 1.2 GHz (measured cold-clock
  MM durations across the whole drain).

kernel(**inputs) takes the FULL unsharded inputs (keys as in the
reference setup_inputs) and returns the FULL [256, 256] output.
"""

import ml_dtypes
import numpy as np

import concourse.bass as bass
import concourse.tile as tile
import concourse.mybir as mybir
from concourse import bacc
from concourse.bass_utils import run_bass_kernel_spmd

F32 = mybir.dt.float32
BF16 = mybir.dt.bfloat16

B_FULL, T_FULL, I_DIM, H, O = 256, 512, 512, 1024, 256
N_CORES = 8
BL = B_FULL // N_CORES  # 32 batch rows per core
KX = I_DIM // 128       # 4 k-tiles of the input dim
KH = H // 128           # 8 k-tiles of the hidden dim
CH = 16                 # chunk length (steps)
LAG = CH + 8            # L1 lags L0 by 24 steps
PREF = 24               # xt DMA prefetch depth (steps ahead)
XS = 2 * CH             # xt ring slots


def _emit_step_mms(nc, ps, stat_tiles, w_sb, kb0, n_k):
    """One recurrence step's s-matmuls, 4x column-tiled.

    ps [128,256] f32 psum; stat_tiles: n_k stationary APs [128,32] bf16;
    w_sb k-block (kb0+i) columns [(kb0+i)*1024 + j] hold W[j, 128*i + r].
    Col-tile v streams columns {j : (j//32)%4 == v} via a strided AP.
    """
    for i in range(n_k):
        blk = w_sb[:, (kb0 + i) * 1024 : (kb0 + i + 1) * 1024].rearrange(
            "p (w f j) -> p w f j", f=4, j=32
        )
        for v in range(4):
            nc.tensor.matmul(
                ps[32 * v : 32 * v + 32, :],
                stat_tiles[i],
                blk[:, :, v : v + 1, :],
                start=(i == 0),
                stop=(i == n_k - 1),
                tile_position=(0, 32 * v),
                skip_group_check=True,
            )


def build_rnn(T):
    assert T % CH == 0
    nc = bacc.Bacc("TRN2", target_bir_lowering=False, debug=False)

    # x is pre-laid-out per step: row block t*128..t*128+128 is step t's
    # tile [128, KX*BL] = x^T in (ki, b) column order -- one contiguous
    # 32 KB block per step -> a single dense DMA burst (the old [I, T*BL]
    # gather produced 64 B scattered elements that crawled when
    # contending with the weight stream).
    xTb_d = nc.dram_tensor("xTb", [T * 128, KX * BL], BF16, kind="ExternalInput").ap()
    w0_d = nc.dram_tensor("w0cat", [128, (KX + KH) * H], BF16, kind="ExternalInput").ap()
    w1_d = nc.dram_tensor("w1cat", [128, (KH + KH) * H], BF16, kind="ExternalInput").ap()
    fcw_d = nc.dram_tensor("fcwT", [128, KH * O], BF16, kind="ExternalInput").ap()
    b0_d = nc.dram_tensor("bias0pp", [128, KH], F32, kind="ExternalInput").ap()
    b1_d = nc.dram_tensor("bias1pp", [128, KH], F32, kind="ExternalInput").ap()
    fcb_d = nc.dram_tensor("fcb", [BL, O], F32, kind="ExternalInput").ap()
    out_d = nc.dram_tensor("out", [BL, O], F32, kind="ExternalOutput").ap()

    with tile.TileContext(nc) as tc:
        with (
            tc.tile_pool(name="wpool", bufs=1) as wpool,
            tc.tile_pool(name="cpool", bufs=1) as cpool,
            tc.tile_pool(name="hT1", bufs=3) as hT1_pool,
            tc.tile_pool(name="tr0", bufs=2) as tr0_pool,
            tc.tile_pool(name="ad0", bufs=2) as ad0_pool,
            tc.tile_pool(name="tr1", bufs=2) as tr1_pool,
            tc.tile_pool(name="ad1", bufs=2) as ad1_pool,
            tc.tile_pool(name="ps0", bufs=2, space="PSUM") as ps0_pool,
            tc.tile_pool(name="ps1", bufs=2, space="PSUM") as ps1_pool,
            tc.tile_pool(name="psc", bufs=3, space="PSUM") as psc_pool,
            tc.tile_pool(name="psh", bufs=1, space="PSUM") as psh_pool,
            tc.tile_pool(name="eout", bufs=1) as eo_pool,
        ):
            w0_sb = wpool.tile([128, (KX + KH) * H], BF16)
            w1_sb = wpool.tile([128, (KH + KH) * H], BF16)
            fcw_sb = wpool.tile([128, KH * O], BF16)
            b0_sb = cpool.tile([128, KH], F32)
            b1_sb = cpool.tile([128, KH], F32)
            fcb_sb = cpool.tile([BL, O], F32)
            # h0T ring: 2*CH slots of [128, 256]; slot u%(2CH) = step u's h0T
            ring = cpool.tile([128, 2 * CH * 256], BF16)
            # pre0T / pre1T double rings: 2 chunks x 16 slots of [128, 256]
            ring0 = cpool.tile([128, 2 * CH * 256], BF16)
            ring2 = cpool.tile([128, 2 * CH * 256], BF16)
            # xt ring: XS slots of [128, 128] (step t -> slot t%XS)
            ringx = cpool.tile([128, XS * 128], BF16)

            # Weight/const DMAs on the gpsimd SWDGE queue; sync queue
            # carries only the xt stream.  Issue order = need order.
            nc.gpsimd.dma_start(b0_sb[:], b0_d)
            nc.gpsimd.dma_start(w0_sb[:, : KX * H], w0_d[:, : KX * H])
            nc.gpsimd.dma_start(w0_sb[:, KX * H : (KX + 4) * H],
                                w0_d[:, KX * H : (KX + 4) * H])
            nc.gpsimd.dma_start(w0_sb[:, (KX + 4) * H :], w0_d[:, (KX + 4) * H :])
            nc.gpsimd.dma_start(b1_sb[:], b1_d)
            nc.gpsimd.dma_start(w1_sb[:, : KH * H], w1_d[:, : KH * H])
            nc.gpsimd.dma_start(w1_sb[:, KH * H :], w1_d[:, KH * H :])
            nc.gpsimd.dma_start(fcw_sb[:], fcw_d)
            nc.gpsimd.dma_start(fcb_sb[:], fcb_d)

            def emit_xt_dma(t, n=1):
                # [t, t+n) must not wrap the ring (n=1 always safe).
                s = t % XS
                nc.sync.dma_start(
                    ringx[:, s * 128 : (s + n) * 128],
                    xTb_d[t * 128 : (t + n) * 128, :].rearrange(
                        "(t p) c -> p t c", p=128
                    ),
                )

            rx_view = ringx.rearrange("p (t k b) -> p t k b", t=XS, b=32)
            r0_view = ring0.rearrange("p (h t k b) -> p h t k b", h=2, t=CH, b=32)
            r2_view = ring2.rearrange("p (h t k b) -> p h t k b", h=2, t=CH, b=32)
            rh_view = ring.rearrange("p (t k b) -> p t k b", t=2 * CH, b=32)

            def x_quarter(c, jb, hc, dve_copy=False):
                """pre0T jb-block for the hc-th half (8 steps) of chunk c:
                x^T chunk from the xt ring streamed against stationary
                W_ih0 jb-blocks; bias0 folded in the copy (ACT in steady
                state; the prologue alternates ACT/DVE so the 16
                back-to-back quarters don't serialize on one engine)."""
                t0 = (c % 2) * CH + 8 * hc
                pc = psc_pool.tile([128, 256], F32, tag="psc")
                for kb in range(KX):
                    nc.tensor.matmul(
                        pc[:, :],
                        w0_sb[:, kb * 1024 + 128 * jb : kb * 1024 + 128 * jb + 128],
                        rx_view[:, t0 : t0 + 8, kb : kb + 1, :],
                        start=(kb == 0),
                        stop=(kb == KX - 1),
                        tile_position=(0, 0),
                        skip_group_check=True,
                    )
                out_ap = r0_view[:, c % 2 : c % 2 + 1, 8 * hc : 8 * hc + 8,
                                 jb : jb + 1, :]
                in_ap = pc[:, :].rearrange("p (t b) -> p t b", b=32)
                if dve_copy:
                    nc.vector.tensor_scalar_add(
                        out_ap, in_ap, b0_sb[:, jb : jb + 1]
                    )
                else:
                    nc.scalar.activation(
                        out_ap, in_ap,
                        mybir.ActivationFunctionType.Identity,
                        bias=b0_sb[:, jb : jb + 1],
                    )

            def phase_c_part(c, jb, hc):
                """pre1T jb-block for the hc-th half (8 steps) of chunk c:
                batched GEMM from the h0T ring (half (c%2)); bias1 folded
                in the PSUM->SBUF copy on ACT."""
                t0 = 8 * hc
                half_base = (c % 2) * CH
                pc = psc_pool.tile([128, 256], F32, tag="psc")
                for kb in range(KH):
                    nc.tensor.matmul(
                        pc[:, :],
                        w1_sb[:, kb * 1024 + 128 * jb : kb * 1024 + 128 * jb + 128],
                        rh_view[:, half_base + t0 : half_base + t0 + 8,
                                kb : kb + 1, :],
                        start=(kb == 0),
                        stop=(kb == KH - 1),
                        tile_position=(0, 0),
                        skip_group_check=True,
                    )
                out_ap = r2_view[:, c % 2 : c % 2 + 1, t0 : t0 + 8, jb : jb + 1, :]
                in_ap = pc[:, :].rearrange("p (t b) -> p t b", b=32)
                nc.scalar.activation(
                    out_ap, in_ap,
                    mybir.ActivationFunctionType.Identity,
                    bias=b1_sb[:, jb : jb + 1],
                )

            # L1-part schedule: part (c, jb, hc) needs h0T of steps
            # c*CH + 8*hc..+7; it runs at step u = c*CH + 8 + 8*hc + jb.
            # All 16 parts of chunk c are done by step c*CH + 23, in time
            # for the first chunk-c L1 step at u = c*CH + LAG.
            def part_for_step(u):
                g = u - 8
                c, r = g // CH, g % CH
                jb, hc = (r, 0) if r < 8 else (r - 8, 1)
                return c, jb, hc

            def l0_step(u, fine=False):
                if u == 0:
                    # h0_0 = relu(pre0_0): no matmul.
                    nc.scalar.activation(
                        ring[:, 0:256],
                        ring0[:, 0:256],
                        mybir.ActivationFunctionType.Relu,
                    )
                    return
                s_prev = 256 * ((u - 1) % (2 * CH))
                prev = ring[:, s_prev : s_prev + 256]
                stats = [prev[:, 32 * k : 32 * k + 32] for k in range(KH)]
                ps0 = ps0_pool.tile([128, 256], F32, tag="ps0")
                _emit_step_mms(nc, ps0, stats, w0_sb, KX, KH)
                # chain: transpose (psum f32 -> sbuf f32) -> +pre0T -> relu
                off0 = ((u // CH) % 2) * (CH * 256) + 256 * (u % CH)
                pre0 = ring0[:, off0 : off0 + 256]
                tr = ring[:, 256 * (u % (2 * CH)) : 256 * (u % (2 * CH)) + 256]
                trt = tr0_pool.tile([128, 256], F32, tag="tr0")
                ad = ad0_pool.tile([128, 256], BF16, tag="ad0")
                W = 64 if fine else 128
                for h in range(256 // W):
                    cs = slice(W * h, W * h + W)
                    nc.vector.transpose(trt[:, cs], ps0[:, cs])
                    nc.vector.tensor_add(ad[:, cs], trt[:, cs], pre0[:, cs])
                    nc.scalar.activation(
                        tr[:, cs], ad[:, cs],
                        mybir.ActivationFunctionType.Relu,
                    )

            hT1 = None

            def l1_step(u, fine=False):
                nonlocal hT1
                off = ((u // CH) % 2) * (CH * 256) + 256 * (u % CH)
                pre1 = ring2[:, off : off + 256]
                if u == 0:
                    hT1_new = hT1_pool.tile([128, 256], BF16, tag="hT1")
                    nc.scalar.activation(
                        hT1_new[:, :], pre1, mybir.ActivationFunctionType.Relu
                    )
                    hT1 = hT1_new
                    return
                stats = [hT1[:, 32 * k : 32 * k + 32] for k in range(KH)]
                ps1 = ps1_pool.tile([128, 256], F32, tag="ps1")
                _emit_step_mms(nc, ps1, stats, w1_sb, KH, KH)
                # chain: transpose -> +pre1T -> relu; fine=True quarters
                # the slices so the serial drain steps expose ~1/4 of the
                # chain latency.
                tr = tr1_pool.tile([128, 256], F32, tag="tr1")
                ad = ad1_pool.tile([128, 256], BF16, tag="ad1")
                hT1_new = hT1_pool.tile([128, 256], BF16, tag="hT1")
                W = 64 if fine else 128
                for h in range(256 // W):
                    cs = slice(W * h, W * h + W)
                    nc.vector.transpose(tr[:, cs], ps1[:, cs])
                    nc.vector.tensor_add(ad[:, cs], tr[:, cs], pre1[:, cs])
                    nc.scalar.activation(
                        hT1_new[:, cs], ad[:, cs],
                        mybir.ActivationFunctionType.Relu,
                    )
                hT1 = hT1_new

            def warm_filler():
                """8 discarded full-array matmuls (~880 ns): keeps HAM from
                re-throttling the PE to 1.2 GHz across the serial drain
                steps and covers the hT1 chain latency like a part would."""
                pw = psc_pool.tile([128, 256], F32, tag="psc")
                for kb in range(KH):
                    nc.tensor.matmul(
                        pw[:, :],
                        w1_sb[:, kb * 1024 : kb * 1024 + 128],
                        w1_sb[:, :256],
                        start=True,
                        stop=True,
                        tile_position=(0, 0),
                        skip_group_check=True,
                    )

            # prologue: prefetch the xt ring (two bulk DMAs -- 24 singles
            # would serialize ~15 us of issue time on the sync queue),
            # then compute chunk 0's pre0 under the weight-DMA window
            # (also warms the PE).
            if T >= PREF:
                emit_xt_dma(0, CH)
                emit_xt_dma(CH, PREF - CH)
            else:
                emit_xt_dma(0, T)
            for q in range(16):
                x_quarter(0, q % 8, q // 8)

            for u in range(T):
                if u + PREF < T:
                    emit_xt_dma(u + PREF)
                l0_step(u, fine=(u < LAG + 2))
                if u >= LAG:
                    l1_step(u - LAG)
                if u >= 8:
                    phase_c_part(*part_for_step(u))
                if u // CH + 1 < T // CH:
                    r = u % CH
                    x_quarter(u // CH + 1, r % 8, r // 8)
            # drain: the last chunk's hc=1 L1-parts interleave with the
            # first 8 tail L1 steps; afterwards discarded filler matmuls
            # keep the PE warm and cover the quartered hT1 chain latency.
            for i, u in enumerate(range(T - LAG, T)):
                l1_step(u, fine=True)
                if i < 8:
                    phase_c_part(T // CH - 1, i, 1)
                elif i < LAG - 1:
                    warm_filler()

            # ---- head: out = h1_last @ fc_w.T + fc_b ----
            hps = psh_pool.tile([BL, O], F32)
            for kb in range(KH):
                nc.tensor.matmul(
                    hps[:, :],
                    hT1[:, 32 * kb : 32 * kb + 32],
                    fcw_sb[:, kb * O : (kb + 1) * O],
                    start=(kb == 0),
                    stop=(kb == KH - 1),
                    tile_position=(0, 0),
                    skip_group_check=True,
                )
            eo = eo_pool.tile([BL, O], F32)
            nc.vector.tensor_add(eo[:, :], hps[:, :], fcb_sb[:, :])
            nc.sync.dma_start(out_d, eo[:, :])

    nc.compile()
    return nc


def _stackT(W, n_k):
    """[128, n_k*cols] bf16: [r, kb*cols + j] = W[j, 128*kb + r]."""
    cols = W.shape[0]
    WT = np.ascontiguousarray(np.asarray(W, np.float32).T)  # [in, out]
    out = np.empty((128, n_k * cols), np.float32)
    for k in range(n_k):
        out[:, k * cols : (k + 1) * cols] = WT[128 * k : 128 * (k + 1), :]
    return out.astype(ml_dtypes.bfloat16)


def _prep_core_inputs(inputs, T):
    f32 = np.float32
    w0cat = np.concatenate(
        [_stackT(np.asarray(inputs["W_ih0"], f32), KX),
         _stackT(np.asarray(inputs["W_hh0"], f32), KH)], axis=1)
    w1cat = np.concatenate(
        [_stackT(np.asarray(inputs["W_ih1"], f32), KH),
         _stackT(np.asarray(inputs["W_hh1"], f32), KH)], axis=1)
    b0 = (np.asarray(inputs["b_ih0"], f32) + np.asarray(inputs["b_hh0"], f32))
    b1 = (np.asarray(inputs["b_ih1"], f32) + np.asarray(inputs["b_hh1"], f32))
    shared = {
        "w0cat": np.ascontiguousarray(w0cat),
        "w1cat": np.ascontiguousarray(w1cat),
        "fcwT": _stackT(np.asarray(inputs["fc_w"], f32), KH),
        "bias0pp": np.ascontiguousarray(b0.reshape(KH, 128).T),
        "bias1pp": np.ascontiguousarray(b1.reshape(KH, 128).T),
        "fcb": np.tile(np.asarray(inputs["fc_b"], f32)[None, :], (BL, 1)),
    }
    x = np.asarray(inputs["input_data"], f32)  # [B, T, I]
    in_maps = []
    for c in range(N_CORES):
        xs = x[c * BL : (c + 1) * BL, :T, :]  # [BL, T, I]
        # [t, p, ki, b]: xT[t*128+p, ki*32+b] = x[b, t, 128*ki+p] -- each
        # step's tile is one contiguous 32 KB block.
        y = xs.transpose(1, 2, 0).reshape(T, KX, 128, BL).transpose(0, 2, 1, 3)
        xT = np.ascontiguousarray(y).reshape(T * 128, KX * BL).astype(
            ml_dtypes.bfloat16)
        in_maps.append(dict(shared, xTb=xT))
    return in_maps


def run(inputs, trace=False, trace_kwargs=None, T=None):
    if T is None:
        T = np.asarray(inputs["input_data"]).shape[1]
    nc = build_rnn(T)
    in_maps = _prep_core_inputs(inputs, T)
    res = run_bass_kernel_spmd(
        nc, in_maps, list(range(N_CORES)), trace=trace, **(trace_kwargs or {})
    )
    out = np.concatenate([res.results[c]["out"] for c in range(N_CORES)], axis=0)
    return out, res


def kernel(**inputs):
    return run(inputs)[0]
